# revision 9
# baseline (speedup 1.0000x reference)
"""FedGATConv forward kernel for Trainium2 (Bass/Tile), 8-core data-parallel.

Computation per node n (N=4096, F=128, S=16, P=9):
  D[n,s]   = att1 . M1[n,:,s] + att2 . M2[n,:,s]
  w[n,p,s] = polycoeffs[p] * D[n,s]^p
  G[n,f]   = sum_{p,s} w[n,p,s] * K1[n,p,s,f]
  E        = G @ weight ; Fden[n] = sum_{p,s} w[n,p,s]*K2[n,p,s]
  out      = E / Fden[:,None]

Sharding: pure data-parallel over nodes, 512 nodes/core, no collectives.

HW model driving the design (all measured on this part):
  - SDMA per-descriptor cost: 512B descs run ~18.7 GB/s/engine (~299 GB/s
    agg), 8KB+ descs ~24 GB/s/engine (HBM-capped ~358 GB/s agg).  K1 in
    natural layout gives 32KB+ descriptors; any transposed (ps-on-partition)
    layout is stuck with 512B descriptors (f is innermost in HBM).
  - fp32 PE matmul is ~4x slower + no FWL -> PE operands must be bf16.
  - DVE tensor_tensor/tensor_reduce fp32 run at 1x (~128 elem/cycle total).

So K1's p-range is split between two pipelines (hybrid):
  - p = 0..NATP (natural path): one big natural-layout DMA per block
    [128 nodes, (NATP+1)*16*128] (40KB contiguous per partition line).
    Weighted sums computed elementwise: DVE handles p in {1,2}, GpSimd
    p in {3,4} and the p=0 plain s-sum; in-place multiply + pairwise-tree
    adds inside the loaded tile, then each reduced [n,f] slice is folded
    into gt via one accumulating PE matmul  gt += slice^T @ (scale*I).
  - p = NATP+1..8 (PE path): transposed load [ps, pair*f] with only
    64 ps values -> two nodes packed per 128-row lhsT (K=64+64), with a
    block-diagonal rhs z built from two PE transposes of w_row.  Loads are
    HWDGE fp32 (512B descs), cast to bf16 on the scalar engine.
  - Fden on DVE in row layout; E = gt.T @ weight; out = E / Fden.
"""

import os
import numpy as np

DEBUG_TAPS = bool(os.environ.get("KERNEL_DEBUG_TAPS"))

N_FULL = 4096
F = 128          # IN_FEAT == OUT_FEAT
S = 16
P = 9
PS = P * S       # 144
NCORES = 8
NS = N_FULL // NCORES   # 512 nodes per core
BLK = 128               # nodes per block
NBLK = NS // BLK        # 4
GRP = 32                # nodes per transposed-K1 DMA group (pairs: GRP/2)
NGRP = BLK // GRP       # groups per block

NATP = 4                # p-values 1..NATP on the natural path (+ p=0)
NATC = (NATP + 1) * S   # leading (p s) columns loaded naturally (incl p=0)
PEP0 = (NATP + 1) * S   # first ps index of the PE path
PEK = PS - PEP0         # contraction length of the PE path (64)

_BUILT = None


def _build():
    """Build and return the compiled Bass module (cached per process)."""
    global _BUILT
    if _BUILT is not None:
        return _BUILT

    import concourse.bacc as bacc
    import concourse.tile as tile
    import concourse.mybir as mybir
    from concourse import masks

    f32 = mybir.dt.float32
    bf16 = mybir.dt.bfloat16
    assert PEK == 64

    nc = bacc.Bacc("TRN2", target_bir_lowering=False, debug=False)

    M1d = nc.dram_tensor("M1", [NS, F, S], f32, kind="ExternalInput").ap()
    M2d = nc.dram_tensor("M2", [NS, F, S], f32, kind="ExternalInput").ap()
    K1d = nc.dram_tensor("K1", [NS, P, S, F], f32, kind="ExternalInput").ap()
    K2d = nc.dram_tensor("K2", [NS, P, S], f32, kind="ExternalInput").ap()
    att1d = nc.dram_tensor("att1", [F], f32, kind="ExternalInput").ap()
    att2d = nc.dram_tensor("att2", [F], f32, kind="ExternalInput").ap()
    wtd = nc.dram_tensor("weight", [F, F], f32, kind="ExternalInput").ap()
    polyd = nc.dram_tensor("polycoeffs", [P], f32, kind="ExternalInput").ap()
    outd = nc.dram_tensor("out", [NS, F], f32, kind="ExternalOutput").ap()
    if DEBUG_TAPS:
        dbg_d = nc.dram_tensor("dbg_d", [NS, S], f32, kind="ExternalOutput").ap()
        dbg_fden = nc.dram_tensor("dbg_fden", [NS, 1], f32, kind="ExternalOutput").ap()
        dbg_gt = nc.dram_tensor("dbg_gt", [NBLK, F, BLK], f32, kind="ExternalOutput").ap()

    K1ps = K1d.rearrange("n p s f -> n (p s) f")   # [NS, 144, 128]
    K2ps = K2d.rearrange("n p s -> n (p s)")       # [NS, 144]

    with tile.TileContext(nc) as tc:
        with (
            tc.tile_pool(name="const", bufs=1) as cpool,
            tc.tile_pool(name="m12", bufs=2) as mpool,
            tc.tile_pool(name="knat", bufs=2) as knatpool,
            tc.tile_pool(name="k1a", bufs=4) as k1apool,
            tc.tile_pool(name="k1abf", bufs=6) as k1abfpool,
            tc.tile_pool(name="small", bufs=2) as spool,
            tc.tile_pool(name="pw", bufs=2) as pwpool,
            tc.tile_pool(name="ps_wt", bufs=1, space="PSUM") as pswt,
            tc.tile_pool(name="ps_gt", bufs=2, space="PSUM") as psgt,
            tc.tile_pool(name="ps_e", bufs=1, space="PSUM") as pse,
        ):
            # ---------------- constants ----------------
            w_sb = cpool.tile([F, F], f32)            # weight [f, o]
            nc.sync.dma_start(w_sb[:], wtd[:])

            ident = cpool.tile([128, 128], f32)
            masks.make_identity(nc, ident[:])

            ones_row = cpool.tile([1, 128], f32)
            nc.vector.memset(ones_row[:], 1.0)

            poly_row = cpool.tile([1, P], f32)
            nc.sync.dma_start(poly_row[:], polyd.unsqueeze(0))
            att1_row = cpool.tile([1, F], f32)
            att2_row = cpool.tile([1, F], f32)
            nc.sync.dma_start(att1_row[:], att1d.unsqueeze(0))
            nc.sync.dma_start(att2_row[:], att2d.unsqueeze(0))

            poly_ps = pse.tile([128, P], f32, tag="polyps")
            nc.tensor.matmul(poly_ps[:], ones_row[:], poly_row[:],
                             start=True, stop=True)
            poly_rep = cpool.tile([128, P], f32)
            nc.vector.tensor_copy(poly_rep[:], poly_ps[:])

            # c0-scaled identity: gt += s0^T @ (c0*I) folds the p=0 term
            c0_ident = cpool.tile([128, 128], f32)
            nc.vector.tensor_scalar(c0_ident[:], ident[:],
                                    poly_rep[:, 0:1], None,
                                    op0=mybir.AluOpType.mult)

            att1_bc = cpool.tile([128, F], f32)
            att2_bc = cpool.tile([128, F], f32)
            for row, bc in ((att1_row, att1_bc), (att2_row, att2_bc)):
                ps_t = pse.tile([128, F], f32, tag="attps")
                nc.tensor.matmul(ps_t[:], ones_row[:], row[:], start=True, stop=True)
                nc.vector.tensor_copy(bc[:], ps_t[:])

            # ---------------- per-block pipeline ----------------
            for blk in range(NBLK):
                nb = blk * BLK

                # -- DMAs: natural-layout loads (big descriptors) --
                m1n = mpool.tile([BLK, F * S], f32, tag="m1")
                m2n = mpool.tile([BLK, F * S], f32, tag="m2")
                nc.sync.dma_start(m1n[:], M1d[nb:nb + BLK].rearrange("n f s -> n (f s)"))
                nc.scalar.dma_start(m2n[:], M2d[nb:nb + BLK].rearrange("n f s -> n (f s)"))

                k2row = spool.tile([BLK, PS], f32, tag="k2")
                nc.sync.dma_start(k2row[:], K2ps[nb:nb + BLK])

                # ps 0..NATC in natural layout: 40KB contiguous per line
                knat = knatpool.tile([BLK, NATC * F], f32)
                nc.scalar.dma_start(
                    knat[:], K1ps[nb:nb + BLK, 0:NATC, :].rearrange("n c f -> n (c f)"))

                # -- PE-path loads: ps PEP0..144 transposed, 2 nodes/row-set --
                # partitions 0..63 <- even nodes, 64..127 <- odd nodes
                k1a_g = []
                for g in range(NGRP):
                    n0 = nb + g * GRP
                    ka = k1apool.tile([128, (GRP // 2) * F], f32)
                    nc.sync.dma_start(
                        ka[0:PEK, :],
                        K1ps[n0:n0 + GRP:2, PEP0:PS, :].transpose([1, 0, 2]))
                    nc.sync.dma_start(
                        ka[PEK:128, :],
                        K1ps[n0 + 1:n0 + GRP:2, PEP0:PS, :].transpose([1, 0, 2]))
                    ka_bf = k1abfpool.tile([128, (GRP // 2) * F], bf16)
                    nc.scalar.copy(ka_bf[:], ka[:])
                    k1a_g.append(ka_bf)

                # -- D on DVE, in-place into m1n/m2n --
                att1_x = att1_bc[:].unsqueeze(2).broadcast_to([BLK, F, S])
                att2_x = att2_bc[:].unsqueeze(2).broadcast_to([BLK, F, S])
                m1v = m1n[:].rearrange("n (f s) -> n f s", s=S)
                m2v = m2n[:].rearrange("n (f s) -> n f s", s=S)
                m1r = m1n[:].rearrange("n (f s) -> n s f", s=S)
                d_ns = spool.tile([BLK, S], f32, tag="dns")
                d_tmp = spool.tile([BLK, S], f32, tag="dtmp")
                nc.vector.tensor_tensor(out=m1v, in0=m1v, in1=att1_x,
                                        op=mybir.AluOpType.mult)
                nc.vector.tensor_reduce(d_tmp[:], m1r,
                                        axis=mybir.AxisListType.X,
                                        op=mybir.AluOpType.add)
                # M2 branch on gpsimd; f-halves are contiguous column ranges in
                # the (f s) layout, so the f-sum is a pairwise in-place tree
                # (gpsimd tensor_reduce only does partition-axis reductions).
                nc.gpsimd.tensor_tensor(out=m2v, in0=m2v, in1=att2_x,
                                        op=mybir.AluOpType.mult)
                fh = (F // 2) * S
                while fh >= S:
                    nc.gpsimd.tensor_tensor(out=m2n[:, 0:fh], in0=m2n[:, 0:fh],
                                            in1=m2n[:, fh:2 * fh],
                                            op=mybir.AluOpType.add)
                    fh //= 2
                nc.vector.tensor_tensor(out=d_ns[:], in0=m2n[:, 0:S],
                                        in1=d_tmp[:],
                                        op=mybir.AluOpType.add)

                # -- powers / w in row layout --
                # w_row col j corresponds to ps = 16 + j  (p = 1..8)
                w_row = spool.tile([BLK, 128], f32, tag="wrow")
                nc.vector.tensor_scalar(w_row[:, 0:S], d_ns[:],
                                        poly_rep[:, 1:2], None,
                                        op0=mybir.AluOpType.mult)
                pcur = d_ns
                for p in range(2, P):
                    pnxt = pwpool.tile([BLK, S], f32, tag="pw")
                    nc.vector.tensor_tensor(out=pnxt[:], in0=pcur[:], in1=d_ns[:],
                                            op=mybir.AluOpType.mult)
                    nc.vector.tensor_scalar(w_row[:, S * (p - 1):S * p], pnxt[:],
                                            poly_rep[:, p:p + 1], None,
                                            op0=mybir.AluOpType.mult)
                    pcur = pnxt

                # -- Fden in row layout: V = w .* K2, reduce over free dim --
                v_row = spool.tile([BLK, PS], f32, tag="vrow")
                nc.vector.tensor_scalar(v_row[:, 0:S], k2row[:, 0:S],
                                        poly_rep[:, 0:1], None,
                                        op0=mybir.AluOpType.mult)
                nc.vector.tensor_tensor(out=v_row[:, S:PS], in0=w_row[:],
                                        in1=k2row[:, S:PS],
                                        op=mybir.AluOpType.mult)
                fden = spool.tile([BLK, 1], f32, tag="fden")
                nc.vector.tensor_reduce(fden[:], v_row[:],
                                        axis=mybir.AxisListType.X,
                                        op=mybir.AluOpType.add)
                rec = spool.tile([BLK, 1], f32, tag="rec")
                nc.vector.reciprocal(rec[:], fden[:])

                # -- natural path: in-place weighted sums inside knat --
                # knat cols: c*F..(c+1)*F holds (p, s) with c = p*16+s.
                # DVE owns p in {1,2}; GpSimd owns p in {3,4} and the p=0 sum.
                def nat_slice(c_lo, n_c):
                    return knat[:, c_lo * F:(c_lo + n_c) * F]

                def nat_view(c_lo, n_c):
                    return nat_slice(c_lo, n_c).rearrange("n (c f) -> n c f", f=F)

                def w_bcast(c_lo, n_c):
                    # w_row col j = ps 16+j -> (p s) col c = j + 16
                    return (w_row[:, c_lo - S:c_lo - S + n_c]
                            .unsqueeze(2).broadcast_to([BLK, n_c, F]))

                for eng, p_lo in ((nc.vector, 1), (nc.gpsimd, 3)):
                    c_lo = p_lo * S
                    # multiply both p-slices by their weights (in place)
                    eng.tensor_tensor(out=nat_view(c_lo, 2 * S),
                                      in0=nat_view(c_lo, 2 * S),
                                      in1=w_bcast(c_lo, 2 * S),
                                      op=mybir.AluOpType.mult)
                    # pairwise tree over the 32 (p,s) slices -> knat[:, c_lo*F:+F]
                    half = S
                    while half >= 1:
                        eng.tensor_tensor(
                            out=nat_slice(c_lo, half),
                            in0=nat_slice(c_lo, half),
                            in1=nat_slice(c_lo + half, half),
                            op=mybir.AluOpType.add)
                        half //= 2
                # p=0 plain s-sum on gpsimd (weight folded via c0_ident)
                half = S // 2
                while half >= 1:
                    nc.gpsimd.tensor_tensor(
                        out=nat_slice(0, half),
                        in0=nat_slice(0, half),
                        in1=nat_slice(half, half),
                        op=mybir.AluOpType.add)
                    half //= 2

                # -- transposes of w for the PE path --
                # T_full rows 64..127 = ps 80..143 weights for odd nodes;
                # T_high rows 0..63   = same weights for even nodes.
                wt_full = pswt.tile([128, 128], f32, tag="wtfull")
                nc.tensor.transpose(wt_full[:], w_row[:], ident[:])
                wt_high = pswt.tile([PEK, 128], f32, tag="wthigh")
                nc.tensor.transpose(wt_high[:], w_row[:, PEK:128], ident[:])

                # block-diagonal rhs z: col b even -> [w_b; 0], odd -> [0; w_b]
                z = spool.tile([128, BLK], bf16, tag="z")
                nc.vector.memset(z[:], 0.0)
                nc.vector.tensor_copy(z[0:PEK, 0:BLK:2], wt_high[:, 0:BLK:2])
                nc.vector.tensor_copy(z[PEK:128, 1:BLK:2], wt_full[PEK:128, 1:BLK:2])

                # -- G accumulation in PSUM [f, node] --
                gt_ps = psgt.tile([128, BLK], f32)
                # p=0 fold (first matmul: carries start=True, full tile)
                nc.tensor.matmul(gt_ps[:], nat_slice(0, 1), c0_ident[:],
                                 start=True, stop=False, skip_group_check=True)
                # natural-path folds: gt += slice^T @ I
                nc.tensor.matmul(gt_ps[:], nat_slice(S, 1), ident[:],
                                 start=False, stop=False, skip_group_check=True)
                nc.tensor.matmul(gt_ps[:], nat_slice(3 * S, 1), ident[:],
                                 start=False, stop=False, skip_group_check=True)
                # PE path: one matmul per node pair (K=64+64, N=2)
                npairs = GRP // 2
                for b in range(0, BLK, 2):
                    g, j = b // GRP, (b % GRP) // 2
                    nc.tensor.matmul(gt_ps[:, b:b + 2],
                                     k1a_g[g][:, j * F:(j + 1) * F],
                                     z[:, b:b + 2],
                                     start=False, stop=(b == BLK - 2),
                                     skip_group_check=True)

                gt_sb = spool.tile([128, BLK], f32, tag="gtsb")
                nc.vector.tensor_copy(gt_sb[:], gt_ps[:])

                if DEBUG_TAPS:
                    nc.sync.dma_start(dbg_d[nb:nb + BLK, :], d_ns[:])
                    nc.sync.dma_start(dbg_fden[nb:nb + BLK, :], fden[:])
                    nc.sync.dma_start(dbg_gt[blk], gt_sb[:])

                # -- E = gt.T @ weight (fp32), scale rows by 1/Fden --
                e_ps = pse.tile([BLK, F], f32)
                nc.tensor.matmul(e_ps[:], gt_sb[:], w_sb[:], start=True, stop=True)
                out_sb = spool.tile([BLK, F], f32, tag="outsb")
                nc.vector.tensor_scalar(out_sb[:], e_ps[:], rec[:], None,
                                        op0=mybir.AluOpType.mult)
                nc.sync.dma_start(outd[nb:nb + BLK, :], out_sb[:])

    nc.compile()
    _BUILT = nc
    return nc


def _run_sharded(inputs, trace=False, trace_kwargs=None):
    """Shard inputs over 8 cores, run, gather. Returns (out, BassKernelResults)."""
    from concourse.bass_utils import run_bass_kernel_spmd

    M1 = np.ascontiguousarray(np.asarray(inputs["M1"], dtype=np.float32))
    M2 = np.ascontiguousarray(np.asarray(inputs["M2"], dtype=np.float32))
    K1 = np.ascontiguousarray(np.asarray(inputs["K1"], dtype=np.float32))
    K2 = np.ascontiguousarray(np.asarray(inputs["K2"], dtype=np.float32))
    att1 = np.ascontiguousarray(np.asarray(inputs["att1"], dtype=np.float32))
    att2 = np.ascontiguousarray(np.asarray(inputs["att2"], dtype=np.float32))
    weight = np.ascontiguousarray(np.asarray(inputs["weight"], dtype=np.float32))
    poly = np.ascontiguousarray(np.asarray(inputs["polycoeffs"], dtype=np.float32))

    nc = _build()
    in_maps = []
    for c in range(NCORES):
        lo, hi = c * NS, (c + 1) * NS
        in_maps.append({
            "M1": M1[lo:hi], "M2": M2[lo:hi],
            "K1": K1[lo:hi], "K2": K2[lo:hi],
            "att1": att1, "att2": att2, "weight": weight,
            "polycoeffs": poly,
        })
    kwargs = {}
    if trace:
        kwargs["trace"] = True
        if trace_kwargs:
            kwargs.update(trace_kwargs)
    res = run_bass_kernel_spmd(nc, in_maps, core_ids=list(range(NCORES)), **kwargs)
    out = np.concatenate([res.results[c]["out"] for c in range(NCORES)], axis=0)
    return out, res


def kernel(**inputs):
    out, _ = _run_sharded(inputs, trace=False)
    return out


# revision 10
# speedup vs baseline: 1.1122x; 1.1122x over previous
"""FedGATConv forward kernel for Trainium2 (Bass/Tile), 8-core data-parallel.

Computation per node n (N=4096, F=128, S=16, P=9):
  D[n,s]   = att1 . M1[n,:,s] + att2 . M2[n,:,s]
  w[n,p,s] = polycoeffs[p] * D[n,s]^p
  G[n,f]   = sum_{p,s} w[n,p,s] * K1[n,p,s,f]
  E        = G @ weight ; Fden[n] = sum_{p,s} w[n,p,s]*K2[n,p,s]
  out      = E / Fden[:,None]

Sharding: pure data-parallel over nodes, 512 nodes/core, no collectives.

HW model driving the design (measured on this part):
  - SDMA per-descriptor cost: 512B descriptors (any ps-on-partition K1
    layout) cap aggregate DMA at ~300 GB/s; natural-layout loads give
    36KB descriptors that reach the ~358 GB/s HBM roofline.  So ALL loads
    here are natural-layout.
  - fp32 PE matmul is ~4x slower + no FWL; PE operands are bf16.
  - With K1 natural ([node, (c f)], c = p*16+s), the weight w[n, c] is a
    PER-PARTITION scalar for fixed c.  So each [128, 128] c-slice is
    scaled by one tensor_scalar (DVE 2-port mode) or scalar-engine mul
    (cast to bf16 in the same op), and the c-reduction is done FOR FREE
    on the otherwise-idle PE by accumulating  gt += slice^T @ I  matmuls
    into PSUM.  No transposed DMA, no DVE reduces over K1.
  - DVE+GpSimd concurrent big ops degrade each other ~2.2x, so gpsimd is
    left idle; the slice scaling is split DVE : ACT = 2 : 1.
"""

import os
import numpy as np

DEBUG_TAPS = bool(os.environ.get("KERNEL_DEBUG_TAPS"))

N_FULL = 4096
F = 128          # IN_FEAT == OUT_FEAT
S = 16
P = 9
PS = P * S       # 144 = number of (p, s) columns c
NCORES = 8
NS = N_FULL // NCORES   # 512 nodes per core
BLK = 128               # nodes per block
NBLK = NS // BLK        # 4
KHALF = PS // 2         # c-columns per K1 half-tile (72)
CCHUNK = 8              # c-slices per scale/fold chunk
NCHUNK = PS // CCHUNK   # 18 chunks per block

_BUILT = None


def _build():
    """Build and return the compiled Bass module (cached per process)."""
    global _BUILT
    if _BUILT is not None:
        return _BUILT

    import concourse.bacc as bacc
    import concourse.tile as tile
    import concourse.mybir as mybir
    from concourse import masks

    f32 = mybir.dt.float32
    bf16 = mybir.dt.bfloat16

    nc = bacc.Bacc("TRN2", target_bir_lowering=False, debug=False)

    M1d = nc.dram_tensor("M1", [NS, F, S], f32, kind="ExternalInput").ap()
    M2d = nc.dram_tensor("M2", [NS, F, S], f32, kind="ExternalInput").ap()
    K1d = nc.dram_tensor("K1", [NS, P, S, F], f32, kind="ExternalInput").ap()
    K2d = nc.dram_tensor("K2", [NS, P, S], f32, kind="ExternalInput").ap()
    att1d = nc.dram_tensor("att1", [F], f32, kind="ExternalInput").ap()
    att2d = nc.dram_tensor("att2", [F], f32, kind="ExternalInput").ap()
    wtd = nc.dram_tensor("weight", [F, F], f32, kind="ExternalInput").ap()
    polyd = nc.dram_tensor("polycoeffs", [P], f32, kind="ExternalInput").ap()
    outd = nc.dram_tensor("out", [NS, F], f32, kind="ExternalOutput").ap()
    if DEBUG_TAPS:
        dbg_d = nc.dram_tensor("dbg_d", [NS, S], f32, kind="ExternalOutput").ap()
        dbg_fden = nc.dram_tensor("dbg_fden", [NS, 1], f32, kind="ExternalOutput").ap()
        dbg_gt = nc.dram_tensor("dbg_gt", [NBLK, F, BLK], f32, kind="ExternalOutput").ap()

    K1ps = K1d.rearrange("n p s f -> n (p s) f")   # [NS, 144, 128]
    K2ps = K2d.rearrange("n p s -> n (p s)")       # [NS, 144]

    with tile.TileContext(nc) as tc:
        with (
            tc.tile_pool(name="const", bufs=1) as cpool,
            tc.tile_pool(name="m12", bufs=2) as mpool,
            tc.tile_pool(name="knatA", bufs=2) as knatApool,
            tc.tile_pool(name="knatB", bufs=2) as knatBpool,
            tc.tile_pool(name="scb", bufs=6) as scpool,
            tc.tile_pool(name="small", bufs=2) as spool,
            tc.tile_pool(name="pw", bufs=2) as pwpool,
            tc.tile_pool(name="ps_gt", bufs=2, space="PSUM") as psgt,
            tc.tile_pool(name="ps_e", bufs=1, space="PSUM") as pse,
        ):
            # ---------------- constants ----------------
            w_sb = cpool.tile([F, F], f32)            # weight [f, o]
            nc.sync.dma_start(w_sb[:], wtd[:])

            ident = cpool.tile([128, 128], f32)
            masks.make_identity(nc, ident[:])
            ident_bf = cpool.tile([128, 128], bf16)
            nc.vector.tensor_copy(ident_bf[:], ident[:])

            ones_row = cpool.tile([1, 128], f32)
            nc.vector.memset(ones_row[:], 1.0)

            poly_row = cpool.tile([1, P], f32)
            nc.sync.dma_start(poly_row[:], polyd.unsqueeze(0))
            att1_row = cpool.tile([1, F], f32)
            att2_row = cpool.tile([1, F], f32)
            nc.sync.dma_start(att1_row[:], att1d.unsqueeze(0))
            nc.sync.dma_start(att2_row[:], att2d.unsqueeze(0))

            poly_ps = pse.tile([128, P], f32, tag="polyps")
            nc.tensor.matmul(poly_ps[:], ones_row[:], poly_row[:],
                             start=True, stop=True)
            poly_rep = cpool.tile([128, P], f32)
            nc.vector.tensor_copy(poly_rep[:], poly_ps[:])

            att1_bc = cpool.tile([128, F], f32)
            att2_bc = cpool.tile([128, F], f32)
            for row, bc in ((att1_row, att1_bc), (att2_row, att2_bc)):
                ps_t = pse.tile([128, F], f32, tag="attps")
                nc.tensor.matmul(ps_t[:], ones_row[:], row[:], start=True, stop=True)
                nc.vector.tensor_copy(bc[:], ps_t[:])

            # ---------------- per-block pipeline ----------------
            for blk in range(NBLK):
                nb = blk * BLK

                # -- DMAs: ALL natural layout, split across both HWDGE rings --
                m1n = mpool.tile([BLK, F * S], f32, tag="m1")
                m2n = mpool.tile([BLK, F * S], f32, tag="m2")
                nc.sync.dma_start(m1n[:], M1d[nb:nb + BLK].rearrange("n f s -> n (f s)"))
                nc.scalar.dma_start(m2n[:], M2d[nb:nb + BLK].rearrange("n f s -> n (f s)"))

                k2row = spool.tile([BLK, PS], f32, tag="k2")
                nc.sync.dma_start(k2row[:], K2ps[nb:nb + BLK])

                # K1 block in two half-tiles, 36KB contiguous per line
                knA = knatApool.tile([BLK, KHALF * F], f32)
                knB = knatBpool.tile([BLK, KHALF * F], f32)
                nc.sync.dma_start(
                    knA[:], K1ps[nb:nb + BLK, 0:KHALF, :].rearrange("n c f -> n (c f)"))
                nc.scalar.dma_start(
                    knB[:], K1ps[nb:nb + BLK, KHALF:PS, :].rearrange("n c f -> n (c f)"))

                # -- D on DVE, in-place into m1n/m2n --
                att1_x = att1_bc[:].unsqueeze(2).broadcast_to([BLK, F, S])
                att2_x = att2_bc[:].unsqueeze(2).broadcast_to([BLK, F, S])
                m1v = m1n[:].rearrange("n (f s) -> n f s", s=S)
                m2v = m2n[:].rearrange("n (f s) -> n f s", s=S)
                m1r = m1n[:].rearrange("n (f s) -> n s f", s=S)
                m2r = m2n[:].rearrange("n (f s) -> n s f", s=S)
                d_ns = spool.tile([BLK, S], f32, tag="dns")
                d_tmp = spool.tile([BLK, S], f32, tag="dtmp")
                nc.vector.tensor_tensor(out=m1v, in0=m1v, in1=att1_x,
                                        op=mybir.AluOpType.mult)
                nc.vector.tensor_reduce(d_tmp[:], m1r,
                                        axis=mybir.AxisListType.X,
                                        op=mybir.AluOpType.add)
                nc.vector.tensor_tensor(out=m2v, in0=m2v, in1=att2_x,
                                        op=mybir.AluOpType.mult)
                nc.vector.tensor_reduce(d_ns[:], m2r,
                                        axis=mybir.AxisListType.X,
                                        op=mybir.AluOpType.add)
                nc.vector.tensor_tensor(out=d_ns[:], in0=d_ns[:], in1=d_tmp[:],
                                        op=mybir.AluOpType.add)

                # -- powers / w in row layout --
                # w_row col j corresponds to c = 16 + j  (p = 1..8)
                w_row = spool.tile([BLK, 128], f32, tag="wrow")
                nc.vector.tensor_scalar(w_row[:, 0:S], d_ns[:],
                                        poly_rep[:, 1:2], None,
                                        op0=mybir.AluOpType.mult)
                pcur = d_ns
                for p in range(2, P):
                    pnxt = pwpool.tile([BLK, S], f32, tag="pw")
                    nc.vector.tensor_tensor(out=pnxt[:], in0=pcur[:], in1=d_ns[:],
                                            op=mybir.AluOpType.mult)
                    nc.vector.tensor_scalar(w_row[:, S * (p - 1):S * p], pnxt[:],
                                            poly_rep[:, p:p + 1], None,
                                            op0=mybir.AluOpType.mult)
                    pcur = pnxt

                # -- Fden in row layout: V = w .* K2, reduce over free dim --
                v_row = spool.tile([BLK, PS], f32, tag="vrow")
                nc.vector.tensor_scalar(v_row[:, 0:S], k2row[:, 0:S],
                                        poly_rep[:, 0:1], None,
                                        op0=mybir.AluOpType.mult)
                nc.vector.tensor_tensor(out=v_row[:, S:PS], in0=w_row[:],
                                        in1=k2row[:, S:PS],
                                        op=mybir.AluOpType.mult)
                fden = spool.tile([BLK, 1], f32, tag="fden")
                nc.vector.tensor_reduce(fden[:], v_row[:],
                                        axis=mybir.AxisListType.X,
                                        op=mybir.AluOpType.add)
                rec = spool.tile([BLK, 1], f32, tag="rec")
                nc.vector.reciprocal(rec[:], fden[:])

                # -- scale c-slices to bf16 (DVE/ACT), fold on PE into gt --
                # scalar for slice c: c0 for p=0, else w_row[:, c-16]
                def c_scalar(c):
                    if c < S:
                        return poly_rep[:, 0:1]
                    return w_row[:, c - S:c - S + 1]

                gt_ps = psgt.tile([128, BLK], f32)
                for ch in range(NCHUNK):
                    c0 = ch * CCHUNK
                    kn = knA if c0 < KHALF else knB
                    base = c0 if c0 < KHALF else c0 - KHALF
                    sc = scpool.tile([BLK, CCHUNK * F], bf16, tag="sc")
                    # every third chunk on the scalar engine, rest on DVE
                    use_act = (ch % 3 == 2)
                    for i in range(CCHUNK):
                        c = c0 + i
                        src = kn[:, (base + i) * F:(base + i + 1) * F]
                        dst = sc[:, i * F:(i + 1) * F]
                        if use_act:
                            nc.scalar.mul(dst, src, c_scalar(c))
                        else:
                            nc.vector.tensor_scalar(dst, src, c_scalar(c), None,
                                                    op0=mybir.AluOpType.mult)
                    for i in range(CCHUNK):
                        c = c0 + i
                        nc.tensor.matmul(gt_ps[:], sc[:, i * F:(i + 1) * F],
                                         ident_bf[:],
                                         start=(c == 0), stop=(c == PS - 1),
                                         skip_group_check=True)

                gt_sb = spool.tile([128, BLK], f32, tag="gtsb")
                nc.vector.tensor_copy(gt_sb[:], gt_ps[:])

                if DEBUG_TAPS:
                    nc.sync.dma_start(dbg_d[nb:nb + BLK, :], d_ns[:])
                    nc.sync.dma_start(dbg_fden[nb:nb + BLK, :], fden[:])
                    nc.sync.dma_start(dbg_gt[blk], gt_sb[:])

                # -- E = gt.T @ weight (fp32), scale rows by 1/Fden --
                e_ps = pse.tile([BLK, F], f32)
                nc.tensor.matmul(e_ps[:], gt_sb[:], w_sb[:], start=True, stop=True)
                out_sb = spool.tile([BLK, F], f32, tag="outsb")
                nc.vector.tensor_scalar(out_sb[:], e_ps[:], rec[:], None,
                                        op0=mybir.AluOpType.mult)
                nc.sync.dma_start(outd[nb:nb + BLK, :], out_sb[:])

    nc.compile()
    _BUILT = nc
    return nc


def _run_sharded(inputs, trace=False, trace_kwargs=None):
    """Shard inputs over 8 cores, run, gather. Returns (out, BassKernelResults)."""
    from concourse.bass_utils import run_bass_kernel_spmd

    M1 = np.ascontiguousarray(np.asarray(inputs["M1"], dtype=np.float32))
    M2 = np.ascontiguousarray(np.asarray(inputs["M2"], dtype=np.float32))
    K1 = np.ascontiguousarray(np.asarray(inputs["K1"], dtype=np.float32))
    K2 = np.ascontiguousarray(np.asarray(inputs["K2"], dtype=np.float32))
    att1 = np.ascontiguousarray(np.asarray(inputs["att1"], dtype=np.float32))
    att2 = np.ascontiguousarray(np.asarray(inputs["att2"], dtype=np.float32))
    weight = np.ascontiguousarray(np.asarray(inputs["weight"], dtype=np.float32))
    poly = np.ascontiguousarray(np.asarray(inputs["polycoeffs"], dtype=np.float32))

    nc = _build()
    in_maps = []
    for c in range(NCORES):
        lo, hi = c * NS, (c + 1) * NS
        in_maps.append({
            "M1": M1[lo:hi], "M2": M2[lo:hi],
            "K1": K1[lo:hi], "K2": K2[lo:hi],
            "att1": att1, "att2": att2, "weight": weight,
            "polycoeffs": poly,
        })
    kwargs = {}
    if trace:
        kwargs["trace"] = True
        if trace_kwargs:
            kwargs.update(trace_kwargs)
    res = run_bass_kernel_spmd(nc, in_maps, core_ids=list(range(NCORES)), **kwargs)
    out = np.concatenate([res.results[c]["out"] for c in range(NCORES)], axis=0)
    return out, res


def kernel(**inputs):
    out, _ = _run_sharded(inputs, trace=False)
    return out
